# revision 26
# baseline (speedup 1.0000x reference)
"""Trainium2 Bass kernel for nn_ConvCapsuleLayer3D.

Self-contained: takes FULL inputs x[32,32,32,8,16], W[16,3,3,1,256], b[16,16,1,1],
returns FULL output [32,30,30,16,16] (fp32). Data-parallel over batch across 8
NeuronCores (4 samples each).

Host prep per core: stripped im2col replicas XA[(kh,kw<2,a)=96, (d,h',w')=7200]
and XB[(kh,kw=2,a)=48, 7200] so conv chunk windows are contiguous stationary
APs; weights pre-permuted to (a_out-major, o-minor) columns; fp16 constants.

Device per sample: conv = 2 accumulating f32r matmuls per (d, chunk) -> votes
V[hw_chunk, (i,a,o)] drained to fp16. Dynamic routing (3 iters, iteration-major
with iter-0 uniform-softmax shortcut) per 120-position chunk in fp16: DVE 2x
tensor_tensor for heavy muls + tree-adds for the i/a reductions, squash row-sums
via a 0/1 broadcast matmul on TensorE, remaining elementwise on GPSIMD/ACT.
"""
import os
import sys

import numpy as np

sys.path.insert(0, "/opt/trn_rl_repo")

# --- problem constants (hardcoded; kernel.py must not read /root/problem) ---
B, H, WD, IC, IA = 32, 32, 32, 8, 16
OC, NA = 16, 16
K = 3
HC, WC = H - K + 1, WD - K + 1       # 30, 30
HW = HC * WC                         # 900
CO = OC * NA                         # 256
NCORES = 8
NSAMP = B // NCORES                  # 4
EPS = 1e-7
ROUTINGS = 3

CP_FULL = 120                        # 4 h-rows per chunk
CHUNKS = [(c, CP_FULL, 4) for c in range(7)] + [(7, 60, 2)]  # (c, cp, nj)
NCH = len(CHUNKS)
SAMP_ELEMS = H * WD * IC * IA        # 131072
DCOL = 1024                          # per-(d,a) column run in x

INPUT_NAMES = ["XA", "XB", "WA", "WB", "B2", "BC"]


def make_in_maps(x, W, b):
    """Host prep: per-core input dicts for run_bass_kernel_spmd."""
    x = np.ascontiguousarray(x, np.float32)
    W = np.ascontiguousarray(W, np.float32)
    b = np.ascontiguousarray(b, np.float32)

    W5 = W[:, :, :, 0, :].reshape(IA, K, K, OC, NA)       # [a,kh,kw,o,ao]
    Wp = W5.transpose(1, 2, 0, 4, 3)                      # [kh,kw,a,ao,o]
    WA = np.ascontiguousarray(Wp[:, :2].reshape(96, CO))
    WB = np.ascontiguousarray(Wp[:, 2].reshape(48, CO))

    bm = b[:, :, 0, 0]                                    # [o,a]
    B2 = np.ascontiguousarray(bm.T.reshape(CO)).astype(np.float16)

    rows = np.arange(CP_FULL) // WC
    BC = (rows[:, None] == rows[None, :]).astype(np.float16)

    a = np.arange(IA)
    d = np.arange(IC)
    hh = np.arange(HC)
    ww = np.arange(WC)

    def block_idx(khv, kwv):
        base = (a[None, :] * DCOL + khv[:, None] * WD + kwv[:, None]).reshape(-1)
        off = (d[:, None, None] * (IA * DCOL) + hh[None, :, None] * WD
               + ww[None, None, :]).reshape(-1)
        return base[:, None] + off[None, :]

    khA = np.repeat(np.arange(K), 2)
    kwA = np.tile(np.arange(2), K)
    idxA = block_idx(khA, kwA)          # [96, 7200]
    idxB = block_idx(np.arange(K), np.full(K, 2))  # [48, 7200]

    in_maps = []
    for k in range(NCORES):
        xs = x[k * NSAMP:(k + 1) * NSAMP].reshape(NSAMP, SAMP_ELEMS)
        XA = xs[:, idxA]
        XB = xs[:, idxB]
        in_maps.append({
            "XA": np.ascontiguousarray(XA),
            "XB": np.ascontiguousarray(XB),
            "WA": WA, "WB": WB, "B2": B2, "BC": np.ascontiguousarray(BC),
        })
    return in_maps


def _build_body(ctx, tc, aps):
    import concourse.bass as bass
    import concourse.mybir as mybir

    nc = tc.nc
    f32 = mybir.dt.float32
    f32r = mybir.dt.float32r
    f16 = mybir.dt.float16
    Alu = mybir.AluOpType
    Act = mybir.ActivationFunctionType
    X = mybir.AxisListType.X

    def pap(t, part, dims, off=0):
        return bass.AP(tensor=t.tensor, offset=t.offset + off,
                       ap=[[t.ap[0][0], part]] + dims)

    reps = int(os.environ.get("KREPS", "1"))

    consts = ctx.enter_context(tc.tile_pool(name="consts", bufs=1))
    ima_pool = ctx.enter_context(tc.tile_pool(name="ima", bufs=1))
    imb_pool = ctx.enter_context(tc.tile_pool(name="imb", bufs=1))
    vpool = ctx.enter_context(tc.tile_pool(name="votes", bufs=1))
    big = ctx.enter_context(tc.tile_pool(name="big", bufs=3))      # vr/va/t1/u1
    mid = ctx.enter_context(tc.tile_pool(name="mid", bufs=3))      # t2/u2/u3/s1
    sm = ctx.enter_context(tc.tile_pool(name="sm", bufs=3))        # e/r/sq/...
    prep = ctx.enter_context(tc.tile_pool(name="prep", bufs=2))
    lpool = ctx.enter_context(tc.tile_pool(name="lp", bufs=2))
    actp = ctx.enter_context(tc.tile_pool(name="actp", bufs=3))
    psum_c = ctx.enter_context(tc.tile_pool(name="psc", bufs=1, space="PSUM"))
    psum_s = ctx.enter_context(tc.tile_pool(name="pss", bufs=2, space="PSUM"))

    # ---- constants ----
    wa = consts.tile([96, CO], f32r, tag="wa")
    nc.sync.dma_start(out=wa[:, :], in_=aps["WA"])
    wb = consts.tile([48, CO], f32r, tag="wb")
    nc.sync.dma_start(out=wb[:, :], in_=aps["WB"])
    bfull = consts.tile([128, CO], f16, tag="bfull")
    nc.sync.dma_start(out=bfull[:, :],
                      in_=bass.AP(tensor=aps["B2"].tensor, offset=0,
                                  ap=[[0, 128], [1, CO]]))
    bc = consts.tile([CP_FULL, CP_FULL], f16, tag="bc")
    nc.sync.dma_start(out=bc[:, :], in_=aps["BC"])
    zero_t = consts.tile([128, 1], f32, tag="zero")
    nc.vector.memset(zero_t[:, :], 0.0)
    eps_t = consts.tile([128, 1], f32, tag="eps")
    nc.vector.memset(eps_t[:, :], EPS)

    xa_ap, xb_ap = aps["XA"], aps["XB"]

    for rep in range(reps):
      for s in range(NSAMP):
        imA = ima_pool.tile([96, IC * HW], f32r, tag="imA")
        imB = imb_pool.tile([48, IC * HW], f32r, tag="imB")
        nc.sync.dma_start(out=imA[:, :], in_=bass.AP(
            tensor=xa_ap.tensor, offset=s * 96 * IC * HW,
            ap=[[IC * HW, 96], [1, IC * HW]]))
        nc.scalar.dma_start(out=imB[:, :], in_=bass.AP(
            tensor=xb_ap.tensor, offset=s * 48 * IC * HW,
            ap=[[IC * HW, 48], [1, IC * HW]]))

        # ---- conv: votes V[c] [cp, (i,a,o)] fp16 ----
        V = {}
        for (c, cp, nj) in CHUNKS:
            pc = psum_c.tile([CP_FULL, IC * CO], f32, tag="pc")
            for d in range(IC):
                off = d * HW + c * CP_FULL
                out_sl = pc[:cp, d * CO:(d + 1) * CO]
                nc.tensor.matmul(out_sl, pap(imA, 96, [[1, cp]], off),
                                 wa[:, :], start=True, stop=False)
                nc.tensor.matmul(out_sl, pap(imB, 48, [[1, cp]], off),
                                 wb[:, :], start=False, stop=True)
            V[c] = vpool.tile([CP_FULL, IC * CO], f16, tag=f"V{c}", name=f"V{c}")
            nc.scalar.copy(out=V[c][:cp, :], in_=pc[:cp, :])

        # ---- routing, iteration-major ----
        L = {}
        pre = {}
        for it in range(ROUTINGS):
            for (c, cp, nj) in CHUNKS:
                if it == 0:
                    t1 = big.tile([CP_FULL, 4 * CO], f16, tag="t1")
                    nc.vector.tensor_tensor(
                        out=pap(t1, cp, [[CO, 4], [1, CO]]),
                        in0=pap(V[c], cp, [[CO, 4], [1, CO]]),
                        in1=pap(V[c], cp, [[CO, 4], [1, CO]], off=4 * CO),
                        op=Alu.add)
                    t2 = mid.tile([CP_FULL, 2 * CO], f16, tag="t2")
                    nc.gpsimd.tensor_tensor(
                        out=pap(t2, cp, [[CO, 2], [1, CO]]),
                        in0=pap(t1, cp, [[CO, 2], [1, CO]]),
                        in1=pap(t1, cp, [[CO, 2], [1, CO]], off=2 * CO),
                        op=Alu.add)
                    s1 = mid.tile([CP_FULL, CO], f16, tag="s1")
                    nc.vector.tensor_tensor(
                        out=s1[:cp, :], in0=t2[:cp, 0:CO], in1=t2[:cp, CO:2 * CO],
                        op=Alu.add)
                    pre[c] = prep.tile([CP_FULL, CO], f16, tag=f"pre{c}",
                                       name=f"pre{c}")
                    nc.vector.scalar_tensor_tensor(
                        out=pre[c][:cp, :], in0=s1[:cp, :], scalar=1.0 / OC,
                        in1=pap(bfull, cp, [[1, CO]]), op0=Alu.mult, op1=Alu.add)
                else:
                    e = sm.tile([CP_FULL, IC * OC], f32, tag="e")
                    nc.scalar.activation(out=e[:cp, :], in_=L[c][:cp, :],
                                         func=Act.Exp, bias=zero_t[:cp, :])
                    ssum = sm.tile([CP_FULL, IC], f32, tag="ssum")
                    nc.vector.tensor_reduce(
                        out=ssum[:cp, :],
                        in_=pap(e, cp, [[OC, IC], [1, OC]]), axis=X, op=Alu.add)
                    rs = sm.tile([CP_FULL, IC], f32, tag="rs")
                    nc.vector.reciprocal(out=rs[:cp, :], in_=ssum[:cp, :])
                    r = sm.tile([CP_FULL, IC * OC], f16, tag="r")
                    nc.gpsimd.tensor_tensor(
                        out=pap(r, cp, [[OC, IC], [1, OC]]),
                        in0=pap(e, cp, [[OC, IC], [1, OC]]),
                        in1=pap(rs, cp, [[1, IC], [0, OC]]),
                        op=Alu.mult)
                    vr = big.tile([CP_FULL, IC * CO], f16, tag="vr")
                    nc.vector.tensor_tensor(
                        out=pap(vr, cp, [[CO, IC], [NA, OC], [1, NA]]),
                        in0=pap(V[c], cp, [[CO, IC], [NA, OC], [1, NA]]),
                        in1=pap(r, cp, [[OC, IC], [0, OC], [1, OC]]),
                        op=Alu.mult)
                    t1 = big.tile([CP_FULL, 4 * CO], f16, tag="t1")
                    nc.vector.tensor_tensor(
                        out=pap(t1, cp, [[CO, 4], [1, CO]]),
                        in0=pap(vr, cp, [[CO, 4], [1, CO]]),
                        in1=pap(vr, cp, [[CO, 4], [1, CO]], off=4 * CO),
                        op=Alu.add)
                    t2 = mid.tile([CP_FULL, 2 * CO], f16, tag="t2")
                    nc.gpsimd.tensor_tensor(
                        out=pap(t2, cp, [[CO, 2], [1, CO]]),
                        in0=pap(t1, cp, [[CO, 2], [1, CO]]),
                        in1=pap(t1, cp, [[CO, 2], [1, CO]], off=2 * CO),
                        op=Alu.add)
                    s1 = mid.tile([CP_FULL, CO], f16, tag="s1")
                    nc.vector.tensor_tensor(
                        out=s1[:cp, :], in0=t2[:cp, 0:CO], in1=t2[:cp, CO:2 * CO],
                        op=Alu.add)
                    pre[c] = prep.tile([CP_FULL, CO], f16, tag=f"pre{c}",
                                       name=f"pre{c}")
                    nc.gpsimd.tensor_tensor(
                        out=pre[c][:cp, :], in0=s1[:cp, :],
                        in1=pap(bfull, cp, [[1, CO]]), op=Alu.add)

            for (c, cp, nj) in CHUNKS:
                sq = sm.tile([CP_FULL, CO], f16, tag="sq")
                nc.scalar.activation(out=sq[:cp, :], in_=pre[c][:cp, :],
                                     func=Act.Square, bias=zero_t[:cp, :])
                s2p = psum_s.tile([CP_FULL, CO], f32, tag="s2p")
                nc.tensor.matmul(s2p[:cp, :], bc[:cp, :cp], sq[:cp, :],
                                 start=True, stop=True)
                s2s = sm.tile([CP_FULL, CO], f32, tag="s2s")
                nc.scalar.copy(out=s2s[:cp, :], in_=s2p[:cp, :])
                sq1 = sm.tile([CP_FULL, CO], f32, tag="sq1")
                nc.scalar.activation(out=sq1[:cp, :], in_=s2s[:cp, :],
                                     func=Act.Sqrt, bias=eps_t[:cp, :])
                den = sm.tile([CP_FULL, CO], f32, tag="den")
                nc.vector.scalar_tensor_tensor(
                    out=den[:cp, :], in0=s2s[:cp, :], scalar=1.0,
                    in1=sq1[:cp, :], op0=Alu.add, op1=Alu.mult)
                rden = sm.tile([CP_FULL, CO], f32, tag="rden")
                nc.vector.reciprocal(out=rden[:cp, :], in_=den[:cp, :])
                scl = sm.tile([CP_FULL, CO], f16, tag="scl")
                nc.gpsimd.tensor_tensor(out=scl[:cp, :], in0=s2s[:cp, :],
                                        in1=rden[:cp, :], op=Alu.mult)
                if it < ROUTINGS - 1:
                    act = sm.tile([CP_FULL, CO], f16, tag="act")
                    nc.gpsimd.tensor_tensor(out=act[:cp, :], in0=pre[c][:cp, :],
                                            in1=scl[:cp, :], op=Alu.mult)
                    va = big.tile([CP_FULL, IC * CO], f16, tag="va")
                    nc.vector.tensor_tensor(
                        out=pap(va, cp, [[CO, IC], [1, CO]]),
                        in0=pap(V[c], cp, [[CO, IC], [1, CO]]),
                        in1=pap(act, cp, [[0, IC], [1, CO]]),
                        op=Alu.mult)
                    u1 = big.tile([CP_FULL, IC * 128], f16, tag="u1")
                    nc.vector.tensor_tensor(
                        out=pap(u1, cp, [[128, IC], [1, 128]]),
                        in0=pap(va, cp, [[CO, IC], [1, 128]]),
                        in1=pap(va, cp, [[CO, IC], [1, 128]], off=128),
                        op=Alu.add)
                    u2 = mid.tile([CP_FULL, IC * 64], f16, tag="u2")
                    nc.gpsimd.tensor_tensor(
                        out=pap(u2, cp, [[64, IC], [1, 64]]),
                        in0=pap(u1, cp, [[128, IC], [1, 64]]),
                        in1=pap(u1, cp, [[128, IC], [1, 64]], off=64),
                        op=Alu.add)
                    u3 = mid.tile([CP_FULL, IC * 32], f16, tag="u3")
                    nc.gpsimd.tensor_tensor(
                        out=pap(u3, cp, [[32, IC], [1, 32]]),
                        in0=pap(u2, cp, [[64, IC], [1, 32]]),
                        in1=pap(u2, cp, [[64, IC], [1, 32]], off=32),
                        op=Alu.add)
                    lnew = lpool.tile([CP_FULL, IC * OC], f16, tag=f"L{c}",
                                      name=f"L{c}")
                    nc.vector.tensor_tensor(
                        out=pap(lnew, cp, [[OC, IC], [1, OC]]),
                        in0=pap(u3, cp, [[32, IC], [1, OC]]),
                        in1=pap(u3, cp, [[32, IC], [1, OC]], off=OC),
                        op=Alu.add)
                    if it > 0:
                        nc.gpsimd.tensor_tensor(out=lnew[:cp, :],
                                                in0=lnew[:cp, :],
                                                in1=L[c][:cp, :], op=Alu.add)
                    L[c] = lnew
                else:
                    actf = actp.tile([CP_FULL, CO], f32, tag="actf",
                                     name="actf")
                    nc.gpsimd.tensor_tensor(
                        out=pap(actf, cp, [[1, NA], [NA, OC]]),
                        in0=pap(pre[c], cp, [[OC, NA], [1, OC]]),
                        in1=pap(scl, cp, [[OC, NA], [1, OC]]),
                        op=Alu.mult)
                    dst = bass.AP(tensor=aps["out"].tensor,
                                  offset=s * (HW * CO) + c * (CP_FULL * CO),
                                  ap=[[CO, cp], [1, CO]])
                    nc.sync.dma_start(out=dst, in_=actf[:cp, :])


_CACHED = None


def _build():
    global _CACHED
    if _CACHED is not None:
        return _CACHED
    from contextlib import ExitStack
    import concourse.bacc as bacc
    import concourse.mybir as mybir
    import concourse.tile as tile

    nc = bacc.Bacc("TRN2", target_bir_lowering=False, debug=False,
                   num_devices=NCORES)
    f32 = mybir.dt.float32
    f32r = mybir.dt.float32r
    f16 = mybir.dt.float16
    xa_t = nc.dram_tensor("XA", [NSAMP, 96, IC * HW], f32r, kind="ExternalInput")
    xb_t = nc.dram_tensor("XB", [NSAMP, 48, IC * HW], f32r, kind="ExternalInput")
    wa_t = nc.dram_tensor("WA", [96, CO], f32r, kind="ExternalInput")
    wb_t = nc.dram_tensor("WB", [48, CO], f32r, kind="ExternalInput")
    b2_t = nc.dram_tensor("B2", [CO], f16, kind="ExternalInput")
    bc_t = nc.dram_tensor("BC", [CP_FULL, CP_FULL], f16, kind="ExternalInput")
    out_t = nc.dram_tensor("out", [NSAMP, HC, WC, OC, NA], f32, kind="ExternalOutput")

    aps = {"XA": xa_t.ap(), "XB": xb_t.ap(), "WA": wa_t.ap(), "WB": wb_t.ap(),
           "B2": b2_t.ap(), "BC": bc_t.ap(), "out": out_t.ap()}
    with nc.allow_low_precision(reason="fp16 routing validated vs reference"):
        with tile.TileContext(nc) as tc:
            with ExitStack() as ctx:
                _build_body(ctx, tc, aps)
    nc.compile()
    _CACHED = nc
    return nc


def run(x, W, b, trace=False):
    from concourse.bass_utils import run_bass_kernel_spmd

    nc = _build()
    in_maps = make_in_maps(x, W, b)
    res = run_bass_kernel_spmd(nc, in_maps, core_ids=list(range(NCORES)),
                               trace=trace)
    out = np.concatenate([res.results[k]["out"] for k in range(NCORES)], axis=0)
    return out.reshape(B, HC, WC, OC, NA), res


def kernel(x, W, b):
    out, _ = run(x, W, b, trace=False)
    return out.astype(np.float32)


if __name__ == "__main__":
    nc = _build()
    print("built ok")
